# revision 7
# baseline (speedup 1.0000x reference)
"""NF4 (bitsandbytes-style) 4-bit quantized embedding lookup on 8 TRN2 NeuronCores.

Reference semantics (per token t with id x_t):
    row   = packed[x_t]                      # [512] uint8, two nf4 codes per byte
    hi    = row >> 4 ; lo = row & 0xF        # nibbles, even/odd output positions
    out_t = codebook[interleave(hi, lo)] * absmax[x_t]   # [1024] float32

Sharding: data-parallel over the batch dim (8 batch rows == 8 cores, 4096
tokens each). The 25 MB table is replicated per core; token rows are fetched
with an indirect (gather) DMA.

Decode strategy (exact fp32): the sorted codebook is evaluated as a staircase
    c[q] = c0 + sum_{k=1..15} d_k * (q >= k)
with increments d_k chosen so every fp32 prefix sum lands exactly on c[k]
(verified at build time). The hi-nibble staircase runs directly on raw bytes
(q_hi >= k  <=>  byte >= 16k); the lo staircase runs on (byte & 15) << 4 so
both share thresholds 16k. Per-term masks are one tensor_scalar op
(is_ge, mult) and accumulate with tensor_tensor adds; the per-row absmax scale
(gathered inline as 4 extra bytes per table row) is applied by the scalar
engine while writing the interleaved halves of the output tile.
"""

import numpy as np

try:
    import concourse.bass as bass
except ImportError:  # pragma: no cover - path fallback for bare containers
    import sys

    sys.path.insert(0, "/opt/trn_rl_repo")
    import concourse.bass as bass

import concourse.tile as tile
from concourse import mybir
from concourse.bass import IndirectOffsetOnAxis
from concourse.bass_utils import run_bass_kernel_spmd

V, D = 50257, 1024
B, S = 8, 4096
PACKB = D // 2          # packed bytes per row
ROWB = PACKB + 4        # + absmax fp32 appended per row
P = 128                 # SBUF partitions (tokens per tile)
N_TOK = S               # tokens per core
NT = N_TOK // P         # tiles per core
N_CORES = 8

AF = mybir.ActivationFunctionType
OP = mybir.AluOpType


def _staircase_increments(codebook: np.ndarray) -> np.ndarray:
    """d_k such that the fp32 running sum c0 + d1 + ... + d_k == codebook[k]
    exactly, for every prefix (nudged by ULPs if plain differences round)."""
    c = codebook.astype(np.float32)
    assert c.shape == (16,)
    assert np.all(np.diff(c.astype(np.float64)) > 0), "codebook must be sorted"
    ds = []
    run = c[0]
    for k in range(1, 16):
        d = np.float32(c[k] - run)
        if np.float32(run + d) != c[k]:
            for _ in range(8):
                d = np.nextafter(d, np.float32(c[k] - run), dtype=np.float32)
                if np.float32(run + d) == c[k]:
                    break
        assert np.float32(run + d) == c[k], f"cannot hit codebook[{k}] exactly"
        ds.append(float(d))
        run = np.float32(run + d)
    return np.array(ds, dtype=np.float32)


_MAX_WAITS = 1  # walrus setupSyncWait rejects instructions with too many waits


def _split_wait_heavy(nc, maxw: int = _MAX_WAITS):
    """Walrus caps the number of semaphore waits a single instruction may
    carry; Tile's kernel-tail drain can exceed it (one wait per DMA sem lane
    still unobserved by the sync engine). Splitting excess waits onto
    preceding same-engine NoOps is semantically identical — a sequencer
    executes its instructions in order, so the waits still all happen
    before the original instruction issues."""
    n = 0
    for fn in nc.m.functions:
        for bb in fn.blocks:
            il = bb.instructions
            if not any(
                i.sync_info is not None and len(i.sync_info.on_wait) > maxw
                for i in il
            ):
                continue
            out = []
            for ins in il:
                si = ins.sync_info
                if si is not None and len(si.on_wait) > maxw:
                    waits = list(si.on_wait)
                    while len(waits) > maxw:
                        chunk, waits = waits[:maxw], waits[maxw:]
                        n += 1
                        out.append(
                            mybir.InstNoOp(
                                name=f"WSPLIT-{n}",
                                engine=ins.engine,
                                bass_nofuse=True,
                                sync_info=mybir.SyncInfo(
                                    on_wait=chunk, on_update=[]
                                ),
                            )
                        )
                    ins.sync_info = mybir.SyncInfo(
                        on_wait=waits, on_update=list(si.on_update)
                    )
                out.append(ins)
            bb.instructions = out


def build_kernel(codebook: np.ndarray, n_tok: int = N_TOK, vocab: int = V):
    """Trace the per-core Bass program (SPMD: same program, per-core inputs)."""
    d_k = _staircase_increments(codebook)
    c0 = float(np.float32(codebook[0]))
    nt = n_tok // P

    nc = bass.Bass()
    idx_d = nc.declare_dram_parameter("idx", [n_tok], mybir.dt.int32, isOutput=False)
    tbl_d = nc.declare_dram_parameter("tbl", [vocab, ROWB], mybir.dt.uint8, isOutput=False)
    out_d = nc.declare_dram_parameter("out", [n_tok, D], mybir.dt.float32, isOutput=True)

    with tile.TileContext(nc) as tc:
        with (
            tc.tile_pool(name="const", bufs=1) as const_pool,
            tc.tile_pool(name="gather", bufs=3) as gpool,
            tc.tile_pool(name="work", bufs=3) as wpool,
            tc.tile_pool(name="acc", bufs=2) as apool,
            tc.tile_pool(name="outp", bufs=2) as opool,
        ):
            # all token ids, one small DMA: SBUF [P, nt], column i = tile i
            idx_sb = const_pool.tile([P, nt], mybir.dt.int32)
            nc.sync.dma_start(
                out=idx_sb[:], in_=idx_d[:].rearrange("(n p) -> p n", p=P)
            )
            # staircase base: acc starts at codebook[0]
            c0_tile = const_pool.tile([P, PACKB], mybir.dt.float32)
            nc.vector.memset(c0_tile[:], c0)

            for i in range(nt):
                g = gpool.tile([P, ROWB], mybir.dt.uint8, tag="g")
                nc.gpsimd.indirect_dma_start(
                    out=g[:],
                    out_offset=None,
                    in_=tbl_d[:, :],
                    in_offset=IndirectOffsetOnAxis(ap=idx_sb[:, i : i + 1], axis=0),
                )
                by = g[:, 0:PACKB]                       # raw bytes: hi staircase
                # absmax -> its own DVE-written tile so the scalar engine's
                # scale read doesn't add a DMA wait to the activation
                a_t = wpool.tile([P, 1], mybir.dt.float32, tag="a")
                nc.vector.tensor_copy(
                    out=a_t[:], in_=g[:, PACKB:ROWB].bitcast(mybir.dt.float32)
                )

                l16 = wpool.tile([P, PACKB], mybir.dt.uint8, tag="l16")
                nc.vector.tensor_scalar(
                    out=l16[:], in0=by, scalar1=15, scalar2=4,
                    op0=OP.bitwise_and, op1=OP.logical_shift_left,
                )

                out_t = opool.tile([P, D], mybir.dt.float32, tag="out")
                for half, src in ((0, by), (1, l16[:])):
                    acc = apool.tile([P, PACKB], mybir.dt.float32, tag=f"acc{half}")
                    m = wpool.tile([P, PACKB], mybir.dt.float32, tag=f"m{half}")
                    nc.vector.tensor_scalar(
                        out=m[:], in0=src, scalar1=float(16.0), scalar2=float(d_k[0]),
                        op0=OP.is_ge, op1=OP.mult,
                    )
                    nc.vector.tensor_tensor(
                        out=acc[:], in0=m[:], in1=c0_tile[:], op=OP.add
                    )
                    for k in range(2, 16):
                        m = wpool.tile([P, PACKB], mybir.dt.float32, tag=f"m{half}")
                        nc.vector.tensor_scalar(
                            out=m[:], in0=src,
                            scalar1=float(16.0 * k), scalar2=float(d_k[k - 1]),
                            op0=OP.is_ge, op1=OP.mult,
                        )
                        nc.vector.tensor_tensor(
                            out=acc[:], in0=acc[:], in1=m[:], op=OP.add
                        )
                    # interleaved write with the exact fp32 absmax scale
                    nc.scalar.activation(
                        out=out_t[:, half:D:2], in_=acc[:],
                        func=AF.Copy, bias=0.0, scale=a_t[:, 0:1],
                    )
                nc.sync.dma_start(out=out_d[i * P : (i + 1) * P, :], in_=out_t[:])

    _split_wait_heavy(nc)
    return nc


_CACHE: dict = {}


def _get_nc(codebook: np.ndarray):
    key = codebook.astype(np.float32).tobytes()
    if key not in _CACHE:
        _CACHE[key] = build_kernel(codebook)
    return _CACHE[key]


def kernel(x, packed, absmax, codebook) -> np.ndarray:
    x = np.asarray(x)
    packed = np.asarray(packed, dtype=np.uint8)
    absmax = np.asarray(absmax, dtype=np.float32)
    codebook = np.asarray(codebook, dtype=np.float32)
    assert x.shape == (B, S) and packed.shape == (V, PACKB) and absmax.shape == (V,)

    # table layout prep: append each row's absmax so one gather fetches both
    tbl = np.empty((V, ROWB), dtype=np.uint8)
    tbl[:, :PACKB] = packed
    tbl[:, PACKB:] = absmax.view(np.uint8).reshape(V, 4)

    idx = np.ascontiguousarray(x.astype(np.int32))  # [8, 4096] -> one row per core

    nc = _get_nc(codebook)
    in_maps = [{"idx": idx[c], "tbl": tbl} for c in range(N_CORES)]
    res = run_bass_kernel_spmd(nc, in_maps, core_ids=list(range(N_CORES)))
    out = np.stack([res.results[c]["out"] for c in range(N_CORES)], axis=0)
    return out.astype(np.float32, copy=False)


# revision 17
# speedup vs baseline: 62.1461x; 62.1461x over previous
"""NF4 (bitsandbytes-style) 4-bit quantized embedding lookup on 8 TRN2 NeuronCores.

Reference semantics (per token t with id x_t):
    row   = packed[x_t]                      # [512] uint8, two nf4 codes per byte
    hi    = row >> 4 ; lo = row & 0xF        # nibbles, even/odd output positions
    out_t = codebook[interleave(hi, lo)] * absmax[x_t]   # [1024] float32

Sharding: data-parallel over the batch dim (8 batch rows == 8 cores, 4096
tokens each). The 25 MB table is replicated per core; token rows are fetched
with an indirect (gather) DMA.

Decode strategy (exact fp32): the sorted codebook is evaluated as a staircase
    c[q] = c0 + sum_{k=1..15} d_k * (q >= k)
with increments d_k chosen so every fp32 prefix sum lands exactly on c[k]
(verified at build time). The hi-nibble staircase runs directly on raw bytes
(q_hi >= k  <=>  byte >= 16k); the lo staircase runs on (byte & 15) << 4 so
both share thresholds 16k. Per-term masks are one tensor_scalar op
(is_ge, mult) and accumulate with tensor_tensor adds; the per-row absmax scale
(gathered inline as 4 extra bytes per table row) is applied by the scalar
engine while writing the interleaved halves of the output tile.
"""

import numpy as np

try:
    import concourse.bass as bass
except ImportError:  # pragma: no cover - path fallback for bare containers
    import sys

    sys.path.insert(0, "/opt/trn_rl_repo")
    import concourse.bass as bass

import concourse.tile as tile
from concourse import mybir
from concourse.bass import IndirectOffsetOnAxis
from concourse.bass_utils import run_bass_kernel_spmd

V, D = 50257, 1024
B, S = 8, 4096
PACKB = D // 2          # packed bytes per row
ROWB = PACKB + 4        # + absmax fp32 appended per row
P = 128                 # SBUF partitions (tokens per tile)
N_TOK = S               # tokens per core
NT = N_TOK // P         # tiles per core
N_CORES = 8

AF = mybir.ActivationFunctionType
OP = mybir.AluOpType


def _staircase_increments(codebook: np.ndarray) -> np.ndarray:
    """d_k such that the fp32 running sum c0 + d1 + ... + d_k == codebook[k]
    exactly, for every prefix (nudged by ULPs if plain differences round)."""
    c = codebook.astype(np.float32)
    assert c.shape == (16,)
    assert np.all(np.diff(c.astype(np.float64)) > 0), "codebook must be sorted"
    ds = []
    run = c[0]
    for k in range(1, 16):
        d = np.float32(c[k] - run)
        if np.float32(run + d) != c[k]:
            for _ in range(8):
                d = np.nextafter(d, np.float32(c[k] - run), dtype=np.float32)
                if np.float32(run + d) == c[k]:
                    break
        assert np.float32(run + d) == c[k], f"cannot hit codebook[{k}] exactly"
        ds.append(float(d))
        run = np.float32(run + d)
    return np.array(ds, dtype=np.float32)


_MAX_WAITS = 1  # walrus setupSyncWait rejects instructions with too many waits


def _split_wait_heavy(nc, maxw: int = _MAX_WAITS):
    """Walrus caps the number of semaphore waits a single instruction may
    carry; Tile's kernel-tail drain can exceed it (one wait per DMA sem lane
    still unobserved by the sync engine). Splitting excess waits onto
    preceding same-engine NoOps is semantically identical — a sequencer
    executes its instructions in order, so the waits still all happen
    before the original instruction issues."""
    n = 0
    for fn in nc.m.functions:
        for bb in fn.blocks:
            il = bb.instructions
            if not any(
                i.sync_info is not None and len(i.sync_info.on_wait) > maxw
                for i in il
            ):
                continue
            out = []
            for ins in il:
                si = ins.sync_info
                if si is not None and len(si.on_wait) > maxw:
                    waits = list(si.on_wait)
                    while len(waits) > maxw:
                        chunk, waits = waits[:maxw], waits[maxw:]
                        n += 1
                        out.append(
                            mybir.InstNoOp(
                                name=f"WSPLIT-{n}",
                                engine=ins.engine,
                                bass_nofuse=True,
                                sync_info=mybir.SyncInfo(
                                    on_wait=chunk, on_update=[]
                                ),
                            )
                        )
                    ins.sync_info = mybir.SyncInfo(
                        on_wait=waits, on_update=list(si.on_update)
                    )
                out.append(ins)
            bb.instructions = out


def build_kernel(codebook: np.ndarray, n_tok: int = N_TOK, vocab: int = V,
                 tune: dict | None = None, split_waits: bool = True):
    """Trace the per-core Bass program (SPMD: same program, per-core inputs)."""
    t = {"g_bufs": 10, "m_bufs": 20, "acc_bufs": 6, "ps_bufs": 3, "out_bufs": 4,
         "cost_dve": 8900.0, "cost_pe": 13500.0, "cost_pool": 19500.0}
    if tune:
        t.update(tune)
    d_k = _staircase_increments(codebook)
    c0 = float(np.float32(codebook[0]))
    nt = n_tok // P

    nc = bass.Bass()
    idx_d = nc.declare_dram_parameter("idx", [n_tok], mybir.dt.int32, isOutput=False)
    tbl_d = nc.declare_dram_parameter("tbl", [vocab, ROWB], mybir.dt.uint8, isOutput=False)
    out_d = nc.declare_dram_parameter("out", [n_tok, D], mybir.dt.float32, isOutput=True)

    from concourse.masks import make_identity

    # Greedy per-chain engine balancer over cost-model ns. A chain placed on
    # engine E costs E its accumulate work AND costs the DVE the 15 mask
    # terms (bf16 4x masks for DVE/Pool chains, fp32 2x mask*d for PE).
    CH_COST = {"dve": t["cost_dve"], "pe": t["cost_pe"], "pool": t["cost_pool"]}
    MASK_COST = {"dve": 2950.0, "pe": 4950.0, "pool": 2950.0}
    load = {"dve": 33000.0 / NT * 0, "pe": 0.0, "pool": 35000.0}  # pool: descgen

    def pick_chain_engine():
        best, best_peak = None, None
        for e in CH_COST:
            trial = dict(load)
            trial[e] += CH_COST[e]
            trial["dve"] += MASK_COST[e]
            peak = max(trial.values())
            if best_peak is None or peak < best_peak:
                best, best_peak = e, peak
        load[best] += CH_COST[best]
        load["dve"] += MASK_COST[best]
        return best

    with tile.TileContext(nc) as tc:
        with (
            tc.tile_pool(name="const", bufs=1) as const_pool,
            tc.tile_pool(name="gather", bufs=t["g_bufs"]) as gpool,
            tc.tile_pool(name="work", bufs=t["m_bufs"]) as wpool,
            tc.tile_pool(name="acc", bufs=t["acc_bufs"]) as apool,
            tc.tile_pool(name="psum", bufs=t["ps_bufs"], space="PSUM") as ppool,
            tc.tile_pool(name="outp", bufs=t["out_bufs"]) as opool,
        ):
            # all token ids, one small DMA: SBUF [P, nt], column i = tile i
            idx_sb = const_pool.tile([P, nt], mybir.dt.int32)
            nc.sync.dma_start(
                out=idx_sb[:], in_=idx_d[:].rearrange("(n p) -> p n", p=P)
            )
            # staircase base: acc starts at codebook[0]
            c0_tile = const_pool.tile([P, PACKB], mybir.dt.float32)
            nc.vector.memset(c0_tile[:], c0)
            ident = const_pool.tile([P, P], mybir.dt.float32)
            make_identity(nc, ident[:])

            for i in range(nt):
                g = gpool.tile([P, ROWB], mybir.dt.uint8, tag="g")
                nc.gpsimd.indirect_dma_start(
                    out=g[:],
                    out_offset=None,
                    in_=tbl_d[:, :],
                    in_offset=IndirectOffsetOnAxis(ap=idx_sb[:, i : i + 1], axis=0),
                )
                by = g[:, 0:PACKB]                       # raw bytes: hi staircase
                # absmax -> its own compute-written tile so the final scaling
                # activation doesn't take a DMA-semaphore wait for it
                a_t = wpool.tile([P, 1], mybir.dt.float32, tag="a")
                nc.any.tensor_copy(
                    out=a_t[:], in_=g[:, PACKB:ROWB].bitcast(mybir.dt.float32)
                )

                l16 = wpool.tile([P, PACKB], mybir.dt.uint8, tag="l16")
                nc.any.tensor_scalar(
                    out=l16[:], in0=by, scalar1=15, scalar2=4,
                    op0=OP.bitwise_and, op1=OP.logical_shift_left,
                )

                out_t = opool.tile([P, D], mybir.dt.float32, tag="out")
                for half, src in ((0, by), (1, l16[:])):
                    eng = pick_chain_engine()
                    if eng == "dve":
                        # raw 0/1 masks in bf16 (DVE 4x mode); the d_k scale
                        # fuses into the accumulate: acc = mask*d_k + acc.
                        # bf16 holds 0/1 exactly and the stt computes in fp32,
                        # so the arithmetic matches the fp32 staircase bit-
                        # for-bit.
                        acc = apool.tile([P, PACKB], mybir.dt.float32,
                                         tag=f"acc{half}")
                        prev = c0_tile
                        for k in range(1, 16):
                            m = wpool.tile([P, PACKB], mybir.dt.bfloat16,
                                           tag=f"mb{half}")
                            nc.vector.tensor_scalar(
                                out=m[:], in0=src, scalar1=float(16.0 * k),
                                scalar2=None, op0=OP.is_ge,
                            )
                            nc.vector.scalar_tensor_tensor(
                                out=acc[:], in0=m[:], scalar=float(d_k[k - 1]),
                                in1=prev[:], op0=OP.mult, op1=OP.add,
                            )
                            prev = acc
                    else:
                        # fp32 mask*d terms on the DVE; accumulated either by
                        # fp32 identity matmuls in PSUM (PE; 1.0*x products
                        # and fp32 adds are exact) or Pool tensor_tensor adds
                        # (the only tensor op in Pool ucode).
                        ms = []
                        for k in range(1, 16):
                            m = wpool.tile([P, PACKB], mybir.dt.float32,
                                           tag=f"m{half}")
                            nc.vector.tensor_scalar(
                                out=m[:], in0=src,
                                scalar1=float(16.0 * k), scalar2=float(d_k[k - 1]),
                                op0=OP.is_ge, op1=OP.mult,
                            )
                            ms.append(m)
                        if eng == "pe":
                            acc = ppool.tile([P, PACKB], mybir.dt.float32,
                                             tag=f"ps{half}")
                            nc.tensor.matmul(out=acc[:], lhsT=ident[:],
                                             rhs=c0_tile[:], start=True,
                                             stop=False)
                            for j, m in enumerate(ms):
                                nc.tensor.matmul(out=acc[:], lhsT=ident[:],
                                                 rhs=m[:], start=False,
                                                 stop=(j == len(ms) - 1))
                        else:
                            acc = apool.tile([P, PACKB], mybir.dt.float32,
                                             tag=f"acc{half}")
                            nc.gpsimd.tensor_tensor(
                                out=acc[:], in0=ms[0][:], in1=c0_tile[:],
                                op=OP.add,
                            )
                            for m in ms[1:]:
                                nc.gpsimd.tensor_tensor(
                                    out=acc[:], in0=acc[:], in1=m[:], op=OP.add
                                )
                    # interleaved write with the exact fp32 absmax scale
                    nc.scalar.activation(
                        out=out_t[:, half:D:2], in_=acc[:],
                        func=AF.Copy, bias=0.0, scale=a_t[:, 0:1],
                    )
                nc.sync.dma_start(out=out_d[i * P : (i + 1) * P, :], in_=out_t[:])

    if split_waits:
        # needed for walrus codegen; CoreSim's race detector rejects the
        # synthetic NoOps, so simulator-based tests build with False
        _split_wait_heavy(nc)
    return nc


_CACHE: dict = {}


def _get_nc(codebook: np.ndarray):
    key = codebook.astype(np.float32).tobytes()
    if key not in _CACHE:
        _CACHE[key] = build_kernel(codebook)
    return _CACHE[key]


def kernel(x, packed, absmax, codebook) -> np.ndarray:
    x = np.asarray(x)
    packed = np.asarray(packed, dtype=np.uint8)
    absmax = np.asarray(absmax, dtype=np.float32)
    codebook = np.asarray(codebook, dtype=np.float32)
    assert x.shape == (B, S) and packed.shape == (V, PACKB) and absmax.shape == (V,)

    # table layout prep: append each row's absmax so one gather fetches both
    tbl = np.empty((V, ROWB), dtype=np.uint8)
    tbl[:, :PACKB] = packed
    tbl[:, PACKB:] = absmax.view(np.uint8).reshape(V, 4)

    idx = np.ascontiguousarray(x.astype(np.int32))  # [8, 4096] -> one row per core

    nc = _get_nc(codebook)
    in_maps = [{"idx": idx[c], "tbl": tbl} for c in range(N_CORES)]
    res = run_bass_kernel_spmd(nc, in_maps, core_ids=list(range(N_CORES)))
    out = np.stack([res.results[c]["out"] for c in range(N_CORES)], axis=0)
    return out.astype(np.float32, copy=False)


# revision 19
# speedup vs baseline: 63.7467x; 1.0258x over previous
"""NF4 (bitsandbytes-style) 4-bit quantized embedding lookup on 8 TRN2 NeuronCores.

Reference semantics (per token t with id x_t):
    row   = packed[x_t]                      # [512] uint8, two nf4 codes per byte
    hi    = row >> 4 ; lo = row & 0xF        # nibbles, even/odd output positions
    out_t = codebook[interleave(hi, lo)] * absmax[x_t]   # [1024] float32

Sharding: data-parallel over the batch dim (8 batch rows == 8 cores, 4096
tokens each). The 25 MB table is replicated per core; token rows are fetched
with an indirect (gather) DMA.

Decode strategy (exact fp32): the sorted codebook is evaluated as a staircase
    c[q] = c0 + sum_{k=1..15} d_k * (q >= k)
with increments d_k chosen so every fp32 prefix sum lands exactly on c[k]
(verified at build time). The hi-nibble staircase runs directly on raw bytes
(q_hi >= k  <=>  byte >= 16k); the lo staircase runs on (byte & 15) << 4 so
both share thresholds 16k. Each per-term mask is one DVE
tensor_scalar op; term accumulation chains are load-balanced across the
vector engine (fused mask*d+acc), GPSIMD (tensor_tensor adds), and the
tensor engine (fp32 identity-matmul accumulation in PSUM) — all bit-exact.
The per-row absmax scale (gathered inline as 4 extra bytes per table row)
is applied by the scalar engine while writing the interleaved halves of the
output tile.
"""

import numpy as np

try:
    import concourse.bass as bass
except ImportError:  # pragma: no cover - path fallback for bare containers
    import sys

    sys.path.insert(0, "/opt/trn_rl_repo")
    import concourse.bass as bass

import concourse.tile as tile
from concourse import mybir
from concourse.bass import IndirectOffsetOnAxis
from concourse.bass_utils import run_bass_kernel_spmd

V, D = 50257, 1024
B, S = 8, 4096
PACKB = D // 2          # packed bytes per row
ROWB = PACKB + 4        # + absmax fp32 appended per row
P = 128                 # SBUF partitions (tokens per tile)
N_TOK = S               # tokens per core
NT = N_TOK // P         # tiles per core
N_CORES = 8

AF = mybir.ActivationFunctionType
OP = mybir.AluOpType


def _staircase_increments(codebook: np.ndarray) -> np.ndarray:
    """d_k such that the fp32 running sum c0 + d1 + ... + d_k == codebook[k]
    exactly, for every prefix (nudged by ULPs if plain differences round)."""
    c = codebook.astype(np.float32)
    assert c.shape == (16,)
    assert np.all(np.diff(c.astype(np.float64)) > 0), "codebook must be sorted"
    ds = []
    run = c[0]
    for k in range(1, 16):
        d = np.float32(c[k] - run)
        if np.float32(run + d) != c[k]:
            for _ in range(8):
                d = np.nextafter(d, np.float32(c[k] - run), dtype=np.float32)
                if np.float32(run + d) == c[k]:
                    break
        assert np.float32(run + d) == c[k], f"cannot hit codebook[{k}] exactly"
        ds.append(float(d))
        run = np.float32(run + d)
    return np.array(ds, dtype=np.float32)


_MAX_WAITS = 1  # walrus setupSyncWait rejects instructions with too many waits


def _split_wait_heavy(nc, maxw: int = _MAX_WAITS):
    """Walrus caps the number of semaphore waits a single instruction may
    carry; Tile's kernel-tail drain can exceed it (one wait per DMA sem lane
    still unobserved by the sync engine). Splitting excess waits onto
    preceding same-engine NoOps is semantically identical — a sequencer
    executes its instructions in order, so the waits still all happen
    before the original instruction issues."""
    n = 0
    for fn in nc.m.functions:
        for bb in fn.blocks:
            il = bb.instructions
            if not any(
                i.sync_info is not None and len(i.sync_info.on_wait) > maxw
                for i in il
            ):
                continue
            out = []
            for ins in il:
                si = ins.sync_info
                if si is not None and len(si.on_wait) > maxw:
                    waits = list(si.on_wait)
                    while len(waits) > maxw:
                        chunk, waits = waits[:maxw], waits[maxw:]
                        n += 1
                        out.append(
                            mybir.InstNoOp(
                                name=f"WSPLIT-{n}",
                                engine=ins.engine,
                                bass_nofuse=True,
                                sync_info=mybir.SyncInfo(
                                    on_wait=chunk, on_update=[]
                                ),
                            )
                        )
                    ins.sync_info = mybir.SyncInfo(
                        on_wait=waits, on_update=list(si.on_update)
                    )
                out.append(ins)
            bb.instructions = out


def build_kernel(codebook: np.ndarray, n_tok: int = N_TOK, vocab: int = V,
                 tune: dict | None = None, split_waits: bool = True):
    """Trace the per-core Bass program (SPMD: same program, per-core inputs)."""
    t = {"g_bufs": 10, "m_bufs": 20, "acc_bufs": 6, "ps_bufs": 3, "out_bufs": 4,
         "cost_dve": 10500.0, "cost_pe": 13500.0, "cost_pool": 18000.0}
    if tune:
        t.update(tune)
    d_k = _staircase_increments(codebook)
    c0 = float(np.float32(codebook[0]))
    nt = n_tok // P

    nc = bass.Bass()
    idx_d = nc.declare_dram_parameter("idx", [n_tok], mybir.dt.int32, isOutput=False)
    tbl_d = nc.declare_dram_parameter("tbl", [vocab, ROWB], mybir.dt.uint8, isOutput=False)
    out_d = nc.declare_dram_parameter("out", [n_tok, D], mybir.dt.float32, isOutput=True)

    from concourse.masks import make_identity

    # Greedy per-chain engine balancer over cost-model ns. A chain placed on
    # engine E costs E its accumulate work AND costs the DVE the 15 mask
    # terms (bf16 4x masks for DVE/Pool chains, fp32 2x mask*d for PE).
    CH_COST = {"dve": t["cost_dve"], "pe": t["cost_pe"], "pool": t["cost_pool"]}
    MASK_COST = {"dve": 2950.0, "pe": 4950.0, "pool": 2950.0}
    load = {"dve": 0.0, "pe": 0.0, "pool": 35000.0}  # pool pre-load: descgen

    def pick_chain_engine():
        best, best_peak = None, None
        for e in CH_COST:
            trial = dict(load)
            trial[e] += CH_COST[e]
            trial["dve"] += MASK_COST[e]
            peak = max(trial.values())
            if best_peak is None or peak < best_peak:
                best, best_peak = e, peak
        load[best] += CH_COST[best]
        load["dve"] += MASK_COST[best]
        return best

    with tile.TileContext(nc) as tc:
        with (
            tc.tile_pool(name="const", bufs=1) as const_pool,
            tc.tile_pool(name="gather", bufs=t["g_bufs"]) as gpool,
            tc.tile_pool(name="work", bufs=t["m_bufs"]) as wpool,
            tc.tile_pool(name="acc", bufs=t["acc_bufs"]) as apool,
            tc.tile_pool(name="psum", bufs=t["ps_bufs"], space="PSUM") as ppool,
            tc.tile_pool(name="outp", bufs=t["out_bufs"]) as opool,
        ):
            # all token ids, one small DMA: SBUF [P, nt], column i = tile i
            idx_sb = const_pool.tile([P, nt], mybir.dt.int32)
            nc.sync.dma_start(
                out=idx_sb[:], in_=idx_d[:].rearrange("(n p) -> p n", p=P)
            )
            # staircase base: acc starts at codebook[0]
            c0_tile = const_pool.tile([P, PACKB], mybir.dt.float32)
            nc.vector.memset(c0_tile[:], c0)
            ident = const_pool.tile([P, P], mybir.dt.float32)
            make_identity(nc, ident[:])

            for i in range(nt):
                g = gpool.tile([P, ROWB], mybir.dt.uint8, tag="g")
                nc.gpsimd.indirect_dma_start(
                    out=g[:],
                    out_offset=None,
                    in_=tbl_d[:, :],
                    in_offset=IndirectOffsetOnAxis(ap=idx_sb[:, i : i + 1], axis=0),
                )
                by = g[:, 0:PACKB]                       # raw bytes: hi staircase
                # absmax -> its own compute-written tile so the final scaling
                # activation doesn't take a DMA-semaphore wait for it
                a_t = wpool.tile([P, 1], mybir.dt.float32, tag="a")
                nc.any.tensor_copy(
                    out=a_t[:], in_=g[:, PACKB:ROWB].bitcast(mybir.dt.float32)
                )

                l16 = wpool.tile([P, PACKB], mybir.dt.uint8, tag="l16")
                nc.any.tensor_scalar(
                    out=l16[:], in0=by, scalar1=15, scalar2=4,
                    op0=OP.bitwise_and, op1=OP.logical_shift_left,
                )

                out_t = opool.tile([P, D], mybir.dt.float32, tag="out")
                for half, src in ((0, by), (1, l16[:])):
                    eng = pick_chain_engine()
                    if eng == "dve":
                        # raw 0/1 masks in bf16 (DVE 4x mode); the d_k scale
                        # fuses into the accumulate: acc = mask*d_k + acc.
                        # bf16 holds 0/1 exactly and the stt computes in fp32,
                        # so the arithmetic matches the fp32 staircase bit-
                        # for-bit.
                        acc = apool.tile([P, PACKB], mybir.dt.float32,
                                         tag=f"acc{half}")
                        prev = c0_tile
                        for k in range(1, 16):
                            m = wpool.tile([P, PACKB], mybir.dt.bfloat16,
                                           tag=f"mb{half}")
                            nc.vector.tensor_scalar(
                                out=m[:], in0=src, scalar1=float(16.0 * k),
                                scalar2=None, op0=OP.is_ge,
                            )
                            nc.vector.scalar_tensor_tensor(
                                out=acc[:], in0=m[:], scalar=float(d_k[k - 1]),
                                in1=prev[:], op0=OP.mult, op1=OP.add,
                            )
                            prev = acc
                    else:
                        # fp32 mask*d terms on the DVE; accumulated either by
                        # fp32 identity matmuls in PSUM (PE; 1.0*x products
                        # and fp32 adds are exact) or Pool tensor_tensor adds
                        # (the only tensor op in Pool ucode).
                        ms = []
                        for k in range(1, 16):
                            m = wpool.tile([P, PACKB], mybir.dt.float32,
                                           tag=f"m{half}")
                            nc.vector.tensor_scalar(
                                out=m[:], in0=src,
                                scalar1=float(16.0 * k), scalar2=float(d_k[k - 1]),
                                op0=OP.is_ge, op1=OP.mult,
                            )
                            ms.append(m)
                        if eng == "pe":
                            acc = ppool.tile([P, PACKB], mybir.dt.float32,
                                             tag=f"ps{half}")
                            nc.tensor.matmul(out=acc[:], lhsT=ident[:],
                                             rhs=c0_tile[:], start=True,
                                             stop=False)
                            for j, m in enumerate(ms):
                                nc.tensor.matmul(out=acc[:], lhsT=ident[:],
                                                 rhs=m[:], start=False,
                                                 stop=(j == len(ms) - 1))
                        else:
                            acc = apool.tile([P, PACKB], mybir.dt.float32,
                                             tag=f"acc{half}")
                            nc.gpsimd.tensor_tensor(
                                out=acc[:], in0=ms[0][:], in1=c0_tile[:],
                                op=OP.add,
                            )
                            for m in ms[1:]:
                                nc.gpsimd.tensor_tensor(
                                    out=acc[:], in0=acc[:], in1=m[:], op=OP.add
                                )
                    # interleaved write with the exact fp32 absmax scale
                    nc.scalar.activation(
                        out=out_t[:, half:D:2], in_=acc[:],
                        func=AF.Copy, bias=0.0, scale=a_t[:, 0:1],
                    )
                nc.sync.dma_start(out=out_d[i * P : (i + 1) * P, :], in_=out_t[:])

    if split_waits:
        # needed for walrus codegen; CoreSim's race detector rejects the
        # synthetic NoOps, so simulator-based tests build with False
        _split_wait_heavy(nc)
    return nc


_CACHE: dict = {}


def _get_nc(codebook: np.ndarray):
    key = codebook.astype(np.float32).tobytes()
    if key not in _CACHE:
        _CACHE[key] = build_kernel(codebook)
    return _CACHE[key]


def kernel(x, packed, absmax, codebook) -> np.ndarray:
    x = np.asarray(x)
    packed = np.asarray(packed, dtype=np.uint8)
    absmax = np.ascontiguousarray(absmax, dtype=np.float32)
    codebook = np.asarray(codebook, dtype=np.float32)
    assert x.shape == (B, S) and packed.shape == (V, PACKB) and absmax.shape == (V,)

    # table layout prep: append each row's absmax so one gather fetches both
    tbl = np.empty((V, ROWB), dtype=np.uint8)
    tbl[:, :PACKB] = packed
    tbl[:, PACKB:] = absmax.view(np.uint8).reshape(V, 4)

    idx = np.ascontiguousarray(x.astype(np.int32))  # [8, 4096] -> one row per core

    nc = _get_nc(codebook)
    in_maps = [{"idx": idx[c], "tbl": tbl} for c in range(N_CORES)]
    res = run_bass_kernel_spmd(nc, in_maps, core_ids=list(range(N_CORES)))
    out = np.stack([res.results[c]["out"] for c in range(N_CORES)], axis=0)
    return out.astype(np.float32, copy=False)


# revision 23
# speedup vs baseline: 64.0286x; 1.0044x over previous
"""NF4 (bitsandbytes-style) 4-bit quantized embedding lookup on 8 TRN2 NeuronCores.

Reference semantics (per token t with id x_t):
    row   = packed[x_t]                      # [512] uint8, two nf4 codes per byte
    hi    = row >> 4 ; lo = row & 0xF        # nibbles, even/odd output positions
    out_t = codebook[interleave(hi, lo)] * absmax[x_t]   # [1024] float32

Sharding: data-parallel over the batch dim (8 batch rows == 8 cores, 4096
tokens each). The 25 MB table is replicated per core; token rows are fetched
with an indirect (gather) DMA.

Decode strategy (exact fp32): the sorted codebook is evaluated as a staircase
    c[q] = c0 + sum_{k=1..15} d_k * (q >= k)
with increments d_k chosen so every fp32 prefix sum lands exactly on c[k]
(verified at build time). The hi-nibble staircase runs directly on raw bytes
(q_hi >= k  <=>  byte >= 16k); the lo staircase runs on (byte & 15) << 4 so
both share thresholds 16k. Each per-term mask is one DVE
tensor_scalar op; term accumulation chains are load-balanced across the
vector engine (fused mask*d+acc), GPSIMD (tensor_tensor adds), and the
tensor engine (fp32 identity-matmul accumulation in PSUM) — all bit-exact.
The per-row absmax scale (gathered inline as 4 extra bytes per table row)
is applied by the scalar engine while writing the interleaved halves of the
output tile.
"""

import numpy as np

try:
    import concourse.bass as bass
except ImportError:  # pragma: no cover - path fallback for bare containers
    import sys

    sys.path.insert(0, "/opt/trn_rl_repo")
    import concourse.bass as bass

import concourse.tile as tile
from concourse import mybir
from concourse.bass import IndirectOffsetOnAxis
from concourse.bass_utils import run_bass_kernel_spmd

V, D = 50257, 1024
B, S = 8, 4096
PACKB = D // 2          # packed bytes per row
ROWB = PACKB + 4        # + absmax fp32 appended per row
P = 128                 # SBUF partitions (tokens per tile)
N_TOK = S               # tokens per core
NT = N_TOK // P         # tiles per core
N_CORES = 8

AF = mybir.ActivationFunctionType
OP = mybir.AluOpType


def _staircase_increments(codebook: np.ndarray) -> np.ndarray:
    """d_k such that the fp32 running sum c0 + d1 + ... + d_k == codebook[k]
    exactly, for every prefix (nudged by ULPs if plain differences round)."""
    c = codebook.astype(np.float32)
    assert c.shape == (16,)
    assert np.all(np.diff(c.astype(np.float64)) > 0), "codebook must be sorted"
    ds = []
    run = c[0]
    for k in range(1, 16):
        d = np.float32(c[k] - run)
        if np.float32(run + d) != c[k]:
            for _ in range(8):
                d = np.nextafter(d, np.float32(c[k] - run), dtype=np.float32)
                if np.float32(run + d) == c[k]:
                    break
        assert np.float32(run + d) == c[k], f"cannot hit codebook[{k}] exactly"
        ds.append(float(d))
        run = np.float32(run + d)
    return np.array(ds, dtype=np.float32)


_MAX_WAITS = 1  # walrus setupSyncWait rejects instructions with too many waits


def _split_wait_heavy(nc, maxw: int = _MAX_WAITS):
    """Walrus caps the number of semaphore waits a single instruction may
    carry; Tile's kernel-tail drain can exceed it (one wait per DMA sem lane
    still unobserved by the sync engine). Splitting excess waits onto
    preceding same-engine NoOps is semantically identical — a sequencer
    executes its instructions in order, so the waits still all happen
    before the original instruction issues."""
    n = 0
    for fn in nc.m.functions:
        for bb in fn.blocks:
            il = bb.instructions
            if not any(
                i.sync_info is not None and len(i.sync_info.on_wait) > maxw
                for i in il
            ):
                continue
            out = []
            for ins in il:
                si = ins.sync_info
                if si is not None and len(si.on_wait) > maxw:
                    waits = list(si.on_wait)
                    while len(waits) > maxw:
                        chunk, waits = waits[:maxw], waits[maxw:]
                        n += 1
                        out.append(
                            mybir.InstNoOp(
                                name=f"WSPLIT-{n}",
                                engine=ins.engine,
                                bass_nofuse=True,
                                sync_info=mybir.SyncInfo(
                                    on_wait=chunk, on_update=[]
                                ),
                            )
                        )
                    ins.sync_info = mybir.SyncInfo(
                        on_wait=waits, on_update=list(si.on_update)
                    )
                out.append(ins)
            bb.instructions = out


def build_kernel(codebook: np.ndarray, n_tok: int = N_TOK, vocab: int = V,
                 tune: dict | None = None, split_waits: bool = True):
    """Trace the per-core Bass program (SPMD: same program, per-core inputs)."""
    t = {"g_bufs": 10, "m_bufs": 22, "acc_bufs": 4, "ps_bufs": 3, "out_bufs": 4,
         "cost_dve": 10500.0, "cost_pe": 13500.0, "cost_pool": 18000.0}
    if tune:
        t.update(tune)
    d_k = _staircase_increments(codebook)
    c0 = float(np.float32(codebook[0]))
    nt = n_tok // P

    nc = bass.Bass()
    idx_d = nc.declare_dram_parameter("idx", [n_tok], mybir.dt.int32, isOutput=False)
    tbl_d = nc.declare_dram_parameter("tbl", [vocab, ROWB], mybir.dt.uint8, isOutput=False)
    out_d = nc.declare_dram_parameter("out", [n_tok, D], mybir.dt.float32, isOutput=True)

    from concourse.masks import make_identity

    # Greedy per-chain engine balancer over cost-model ns. A chain placed on
    # engine E costs E its accumulate work AND costs the DVE the 15 mask
    # terms (bf16 4x masks for DVE chains, fp32 2x mask*d for PE/Pool).
    CH_COST = {"dve": t["cost_dve"], "pe": t["cost_pe"], "pool": t["cost_pool"]}
    MASK_COST = {"dve": 2950.0, "pe": 4950.0, "pool": 2950.0}
    load = {"dve": 0.0, "pe": 0.0, "pool": 35000.0}  # pool pre-load: descgen

    def pick_chain_engine():
        best, best_peak = None, None
        for e in CH_COST:
            trial = dict(load)
            trial[e] += CH_COST[e]
            trial["dve"] += MASK_COST[e]
            peak = max(trial.values())
            if best_peak is None or peak < best_peak:
                best, best_peak = e, peak
        load[best] += CH_COST[best]
        load["dve"] += MASK_COST[best]
        return best

    with tile.TileContext(nc) as tc:
        with (
            tc.tile_pool(name="const", bufs=1) as const_pool,
            tc.tile_pool(name="gather", bufs=t["g_bufs"]) as gpool,
            tc.tile_pool(name="work", bufs=t["m_bufs"]) as wpool,
            tc.tile_pool(name="acc", bufs=t["acc_bufs"]) as apool,
            tc.tile_pool(name="psum", bufs=t["ps_bufs"], space="PSUM") as ppool,
            tc.tile_pool(name="outp", bufs=t["out_bufs"]) as opool,
        ):
            # all token ids, one small DMA: SBUF [P, nt], column i = tile i
            idx_sb = const_pool.tile([P, nt], mybir.dt.int32)
            nc.sync.dma_start(
                out=idx_sb[:], in_=idx_d[:].rearrange("(n p) -> p n", p=P)
            )
            # staircase base: acc starts at codebook[0]
            c0_tile = const_pool.tile([P, PACKB], mybir.dt.float32)
            nc.vector.memset(c0_tile[:], c0)
            ident = const_pool.tile([P, P], mybir.dt.float32)
            make_identity(nc, ident[:])

            for i in range(nt):
                g = gpool.tile([P, ROWB], mybir.dt.uint8, tag="g")
                nc.gpsimd.indirect_dma_start(
                    out=g[:],
                    out_offset=None,
                    in_=tbl_d[:, :],
                    in_offset=IndirectOffsetOnAxis(ap=idx_sb[:, i : i + 1], axis=0),
                )
                by = g[:, 0:PACKB]                       # raw bytes: hi staircase
                # absmax -> its own compute-written tile so the final scaling
                # activation doesn't take a DMA-semaphore wait for it
                a_t = wpool.tile([P, 1], mybir.dt.float32, tag="a")
                nc.any.tensor_copy(
                    out=a_t[:], in_=g[:, PACKB:ROWB].bitcast(mybir.dt.float32)
                )

                l16 = wpool.tile([P, PACKB], mybir.dt.uint8, tag="l16")
                nc.any.tensor_scalar(
                    out=l16[:], in0=by, scalar1=15, scalar2=4,
                    op0=OP.bitwise_and, op1=OP.logical_shift_left,
                )

                out_t = opool.tile([P, D], mybir.dt.float32, tag="out")
                for half, src in ((0, by), (1, l16[:])):
                    eng = pick_chain_engine()
                    if eng == "dve":
                        # raw 0/1 masks in bf16 (DVE 4x mode); the d_k scale
                        # fuses into the accumulate: acc = mask*d_k + acc.
                        # bf16 holds 0/1 exactly and the stt computes in fp32,
                        # so the arithmetic matches the fp32 staircase bit-
                        # for-bit.
                        acc = apool.tile([P, PACKB], mybir.dt.float32,
                                         tag=f"acc{half}")
                        prev = c0_tile
                        for k in range(1, 16):
                            m = wpool.tile([P, PACKB], mybir.dt.bfloat16,
                                           tag=f"mb{half}")
                            nc.vector.tensor_scalar(
                                out=m[:], in0=src, scalar1=float(16.0 * k),
                                scalar2=None, op0=OP.is_ge,
                            )
                            nc.vector.scalar_tensor_tensor(
                                out=acc[:], in0=m[:], scalar=float(d_k[k - 1]),
                                in1=prev[:], op0=OP.mult, op1=OP.add,
                            )
                            prev = acc
                    else:
                        # fp32 mask*d terms on the DVE; accumulated either by
                        # fp32 identity matmuls in PSUM (PE; 1.0*x products
                        # and fp32 adds are exact) or Pool tensor_tensor adds
                        # (the only tensor op in Pool ucode).
                        ms = []
                        for k in range(1, 16):
                            m = wpool.tile([P, PACKB], mybir.dt.float32,
                                           tag=f"m{half}")
                            nc.vector.tensor_scalar(
                                out=m[:], in0=src,
                                scalar1=float(16.0 * k), scalar2=float(d_k[k - 1]),
                                op0=OP.is_ge, op1=OP.mult,
                            )
                            ms.append(m)
                        if eng == "pe":
                            acc = ppool.tile([P, PACKB], mybir.dt.float32,
                                             tag=f"ps{half}")
                            nc.tensor.matmul(out=acc[:], lhsT=ident[:],
                                             rhs=c0_tile[:], start=True,
                                             stop=False)
                            for j, m in enumerate(ms):
                                nc.tensor.matmul(out=acc[:], lhsT=ident[:],
                                                 rhs=m[:], start=False,
                                                 stop=(j == len(ms) - 1))
                        else:
                            acc = apool.tile([P, PACKB], mybir.dt.float32,
                                             tag=f"acc{half}")
                            nc.gpsimd.tensor_tensor(
                                out=acc[:], in0=ms[0][:], in1=c0_tile[:],
                                op=OP.add,
                            )
                            for m in ms[1:]:
                                nc.gpsimd.tensor_tensor(
                                    out=acc[:], in0=acc[:], in1=m[:], op=OP.add
                                )
                    # interleaved write with the exact fp32 absmax scale
                    nc.scalar.activation(
                        out=out_t[:, half:D:2], in_=acc[:],
                        func=AF.Copy, bias=0.0, scale=a_t[:, 0:1],
                    )
                nc.sync.dma_start(out=out_d[i * P : (i + 1) * P, :], in_=out_t[:])

    if split_waits:
        # needed for walrus codegen; CoreSim's race detector rejects the
        # synthetic NoOps, so simulator-based tests build with False
        _split_wait_heavy(nc)
    return nc


_CACHE: dict = {}


def _get_nc(codebook: np.ndarray):
    key = codebook.astype(np.float32).tobytes()
    if key not in _CACHE:
        _CACHE[key] = build_kernel(codebook)
    return _CACHE[key]


def kernel(x, packed, absmax, codebook) -> np.ndarray:
    x = np.asarray(x)
    packed = np.asarray(packed, dtype=np.uint8)
    absmax = np.ascontiguousarray(absmax, dtype=np.float32)
    codebook = np.asarray(codebook, dtype=np.float32)
    assert x.shape == (B, S) and packed.shape == (V, PACKB) and absmax.shape == (V,)

    # table layout prep: append each row's absmax so one gather fetches both
    tbl = np.empty((V, ROWB), dtype=np.uint8)
    tbl[:, :PACKB] = packed
    tbl[:, PACKB:] = absmax.view(np.uint8).reshape(V, 4)

    idx = np.ascontiguousarray(x.astype(np.int32))  # [8, 4096] -> one row per core

    nc = _get_nc(codebook)
    in_maps = [{"idx": idx[c], "tbl": tbl} for c in range(N_CORES)]
    res = run_bass_kernel_spmd(nc, in_maps, core_ids=list(range(N_CORES)))
    out = np.stack([res.results[c]["out"] for c in range(N_CORES)], axis=0)
    return out.astype(np.float32, copy=False)


# revision 26
# speedup vs baseline: 74.6586x; 1.1660x over previous
"""NF4 (bitsandbytes-style) 4-bit quantized embedding lookup on 8 TRN2 NeuronCores.

Reference semantics (per token t with id x_t):
    row   = packed[x_t]                      # [512] uint8, two nf4 codes per byte
    hi    = row >> 4 ; lo = row & 0xF        # nibbles, even/odd output positions
    out_t = codebook[interleave(hi, lo)] * absmax[x_t]   # [1024] float32

Sharding: data-parallel over the batch dim (8 batch rows == 8 cores, 4096
tokens each). The 25 MB table is replicated per core; token rows are fetched
with an indirect (gather) DMA.

Decode strategy (exact fp32): the sorted codebook is evaluated as a staircase
    c[q] = c0 + sum_{k=1..15} d_k * (q >= k)
with increments d_k chosen so every fp32 prefix sum lands exactly on c[k]
(verified at build time). The hi-nibble staircase runs directly on raw bytes
(q_hi >= k  <=>  byte >= 16k); the lo staircase runs on (byte & 15) << 4 so
both share thresholds 16k. Each per-term mask is one DVE
tensor_scalar op; term accumulation chains are load-balanced across the
vector engine (fused mask*d+acc), GPSIMD (tensor_tensor adds), and the
tensor engine (PSUM chains of diag(piece) @ mask matmuls, with increments
pre-split into bf16-exact pieces so every product and add is exact) —
all bit-exact.
The per-row absmax scale (gathered inline as 4 extra bytes per table row)
is applied by the scalar engine while writing the interleaved halves of the
output tile.
"""

import numpy as np

try:
    import concourse.bass as bass
except ImportError:  # pragma: no cover - path fallback for bare containers
    import sys

    sys.path.insert(0, "/opt/trn_rl_repo")
    import concourse.bass as bass

import concourse.tile as tile
from concourse import mybir
from concourse.bass import IndirectOffsetOnAxis
from concourse.bass_utils import run_bass_kernel_spmd

V, D = 50257, 1024
B, S = 8, 4096
PACKB = D // 2          # packed bytes per row
ROWB = PACKB + 4        # + absmax fp32 appended per row
P = 128                 # SBUF partitions (tokens per tile)
N_TOK = S               # tokens per core
NT = N_TOK // P         # tiles per core
N_CORES = 8

AF = mybir.ActivationFunctionType
OP = mybir.AluOpType


def _staircase_increments(codebook: np.ndarray) -> np.ndarray:
    """d_k such that the fp32 running sum c0 + d1 + ... + d_k == codebook[k]
    exactly, for every prefix (nudged by ULPs if plain differences round)."""
    c = codebook.astype(np.float32)
    assert c.shape == (16,)
    assert np.all(np.diff(c.astype(np.float64)) > 0), "codebook must be sorted"
    ds = []
    run = c[0]
    for k in range(1, 16):
        d = np.float32(c[k] - run)
        if np.float32(run + d) != c[k]:
            for _ in range(8):
                d = np.nextafter(d, np.float32(c[k] - run), dtype=np.float32)
                if np.float32(run + d) == c[k]:
                    break
        assert np.float32(run + d) == c[k], f"cannot hit codebook[{k}] exactly"
        ds.append(float(d))
        run = np.float32(run + d)
    return np.array(ds, dtype=np.float32)


def _staircase_bf16_pieces(codebook: np.ndarray) -> list[list[float]]:
    """Split each staircase increment into <=4 bf16-exact pieces such that the
    fp32 running sum (starting at 0, pieces added in order) lands exactly on
    codebook[k] after k's pieces. bf16 x bf16 products are exact in fp32, so
    a PE matmul chain of diag(piece) @ mask accumulates these bit-exactly."""
    import ml_dtypes

    def bf16(x):
        return np.float32(ml_dtypes.bfloat16(np.float32(x)))

    c = codebook.astype(np.float32)
    out: list[list[float]] = []
    S = np.float32(0.0)
    for k in range(16):
        pieces: list[np.float32] = []
        for _ in range(4):
            if S == c[k]:
                break
            p = bf16(np.float32(c[k] - S))
            if p == 0.0:
                break
            pieces.append(p)
            S = np.float32(S + p)
        if S != c[k]:
            for _ in range(8):
                last = pieces[-1]
                Sbase = np.float32(S - last)
                b = ml_dtypes.bfloat16(last)
                bn = np.nextafter(b, ml_dtypes.bfloat16(np.float32(c[k]) - Sbase))
                pieces[-1] = np.float32(bn)
                S = np.float32(Sbase + pieces[-1])
                if S == c[k]:
                    break
                p = bf16(np.float32(c[k] - S))
                if p != 0.0:
                    pieces.append(p)
                    S = np.float32(S + p)
                if S == c[k]:
                    break
            assert S == c[k], f"bf16 piece split failed at codebook[{k}]"
        out.append([float(p) for p in pieces])
    # verify every prefix
    S = np.float32(0.0)
    for k in range(16):
        for p in out[k]:
            S = np.float32(S + p)
        assert S == c[k]
    return out


_MAX_WAITS = 1  # walrus setupSyncWait rejects instructions with too many waits


def _split_wait_heavy(nc, maxw: int = _MAX_WAITS):
    """Walrus caps the number of semaphore waits a single instruction may
    carry; Tile's kernel-tail drain can exceed it (one wait per DMA sem lane
    still unobserved by the sync engine). Splitting excess waits onto
    preceding same-engine NoOps is semantically identical — a sequencer
    executes its instructions in order, so the waits still all happen
    before the original instruction issues."""
    n = 0
    for fn in nc.m.functions:
        for bb in fn.blocks:
            il = bb.instructions
            if not any(
                i.sync_info is not None and len(i.sync_info.on_wait) > maxw
                for i in il
            ):
                continue
            out = []
            for ins in il:
                si = ins.sync_info
                if si is not None and len(si.on_wait) > maxw:
                    waits = list(si.on_wait)
                    while len(waits) > maxw:
                        chunk, waits = waits[:maxw], waits[maxw:]
                        n += 1
                        out.append(
                            mybir.InstNoOp(
                                name=f"WSPLIT-{n}",
                                engine=ins.engine,
                                bass_nofuse=True,
                                sync_info=mybir.SyncInfo(
                                    on_wait=chunk, on_update=[]
                                ),
                            )
                        )
                    ins.sync_info = mybir.SyncInfo(
                        on_wait=waits, on_update=list(si.on_update)
                    )
                out.append(ins)
            bb.instructions = out


def build_kernel(codebook: np.ndarray, n_tok: int = N_TOK, vocab: int = V,
                 tune: dict | None = None, split_waits: bool = True):
    """Trace the per-core Bass program (SPMD: same program, per-core inputs)."""
    t = {"g_bufs": 10, "m_bufs": 22, "acc_bufs": 4, "ps_bufs": 3, "out_bufs": 4,
         "cost_dve": 11000.0, "cost_pe": 8000.0, "cost_pool": 17000.0}
    if tune:
        t.update(tune)
    d_k = _staircase_increments(codebook)
    pieces = _staircase_bf16_pieces(codebook)
    c0 = float(np.float32(codebook[0]))
    nt = n_tok // P

    nc = bass.Bass()
    idx_d = nc.declare_dram_parameter("idx", [n_tok], mybir.dt.int32, isOutput=False)
    tbl_d = nc.declare_dram_parameter("tbl", [vocab, ROWB], mybir.dt.uint8, isOutput=False)
    out_d = nc.declare_dram_parameter("out", [n_tok, D], mybir.dt.float32, isOutput=True)

    from concourse.masks import make_identity

    # Greedy per-chain engine balancer over cost-model ns. A chain placed on
    # engine E costs E its accumulate work AND costs the DVE the 15 mask
    # terms (bf16 4x masks for DVE chains, fp32 2x mask*d for PE/Pool).
    CH_COST = {"dve": t["cost_dve"], "pe": t["cost_pe"], "pool": t["cost_pool"]}
    MASK_COST = {"dve": 2950.0, "pe": 2950.0, "pool": 4950.0}
    load = {"dve": 0.0, "pe": 0.0, "pool": 35000.0}  # pool pre-load: descgen

    def pick_chain_engine():
        best, best_peak = None, None
        for e in CH_COST:
            trial = dict(load)
            trial[e] += CH_COST[e]
            trial["dve"] += MASK_COST[e]
            peak = max(trial.values())
            if best_peak is None or peak < best_peak:
                best, best_peak = e, peak
        load[best] += CH_COST[best]
        load["dve"] += MASK_COST[best]
        return best

    with tile.TileContext(nc) as tc:
        with (
            tc.tile_pool(name="const", bufs=1) as const_pool,
            tc.tile_pool(name="gather", bufs=t["g_bufs"]) as gpool,
            tc.tile_pool(name="work", bufs=t["m_bufs"]) as wpool,
            tc.tile_pool(name="acc", bufs=t["acc_bufs"]) as apool,
            tc.tile_pool(name="psum", bufs=t["ps_bufs"], space="PSUM") as ppool,
            tc.tile_pool(name="outp", bufs=t["out_bufs"]) as opool,
        ):
            # all token ids, one small DMA: SBUF [P, nt], column i = tile i
            idx_sb = const_pool.tile([P, nt], mybir.dt.int32)
            nc.sync.dma_start(
                out=idx_sb[:], in_=idx_d[:].rearrange("(n p) -> p n", p=P)
            )
            # staircase base: acc starts at codebook[0]
            c0_tile = const_pool.tile([P, PACKB], mybir.dt.float32)
            nc.vector.memset(c0_tile[:], c0)
            ident_bf = const_pool.tile([P, P], mybir.dt.bfloat16)
            make_identity(nc, ident_bf[:])
            ones_bf = const_pool.tile([P, PACKB], mybir.dt.bfloat16)
            nc.vector.memset(ones_bf[:], 1.0)
            # diag(piece) lhsT tiles, built by the (otherwise idle) scalar
            # engine: Copy with an fp32-imm scale of a bf16-exact value
            # round-trips exactly through the bf16 output
            diag_tiles = []
            for k in range(16):
                row = []
                for p in pieces[k]:
                    dt_ = const_pool.tile([P, P], mybir.dt.bfloat16,
                                          tag=f"diag{len(diag_tiles)}_{len(row)}")
                    nc.scalar.activation(out=dt_[:], in_=ident_bf[:],
                                         func=AF.Copy, bias=0.0, scale=float(p))
                    row.append(dt_)
                diag_tiles.append(row)

            for i in range(nt):
                g = gpool.tile([P, ROWB], mybir.dt.uint8, tag="g")
                nc.gpsimd.indirect_dma_start(
                    out=g[:],
                    out_offset=None,
                    in_=tbl_d[:, :],
                    in_offset=IndirectOffsetOnAxis(ap=idx_sb[:, i : i + 1], axis=0),
                )
                by = g[:, 0:PACKB]                       # raw bytes: hi staircase
                # absmax -> its own compute-written tile so the final scaling
                # activation doesn't take a DMA-semaphore wait for it
                a_t = wpool.tile([P, 1], mybir.dt.float32, tag="a")
                nc.any.tensor_copy(
                    out=a_t[:], in_=g[:, PACKB:ROWB].bitcast(mybir.dt.float32)
                )

                l16 = wpool.tile([P, PACKB], mybir.dt.uint8, tag="l16")
                nc.any.tensor_scalar(
                    out=l16[:], in0=by, scalar1=15, scalar2=4,
                    op0=OP.bitwise_and, op1=OP.logical_shift_left,
                )

                out_t = opool.tile([P, D], mybir.dt.float32, tag="out")
                for half, src in ((0, by), (1, l16[:])):
                    eng = pick_chain_engine()
                    if eng == "dve":
                        # raw 0/1 masks in bf16 (DVE 4x mode); the d_k scale
                        # fuses into the accumulate: acc = mask*d_k + acc.
                        # bf16 holds 0/1 exactly and the stt computes in fp32,
                        # so the arithmetic matches the fp32 staircase bit-
                        # for-bit.
                        acc = apool.tile([P, PACKB], mybir.dt.float32,
                                         tag=f"acc{half}")
                        prev = c0_tile
                        for k in range(1, 16):
                            m = wpool.tile([P, PACKB], mybir.dt.bfloat16,
                                           tag=f"mb{half}")
                            nc.vector.tensor_scalar(
                                out=m[:], in0=src, scalar1=float(16.0 * k),
                                scalar2=None, op0=OP.is_ge,
                            )
                            nc.vector.scalar_tensor_tensor(
                                out=acc[:], in0=m[:], scalar=float(d_k[k - 1]),
                                in1=prev[:], op0=OP.mult, op1=OP.add,
                            )
                            prev = acc
                    elif eng == "pe":
                        # all-bf16 PSUM chain: raw 0/1 masks (bf16, DVE 4x)
                        # and diag(piece) weights; every product/add is exact
                        # and the piece order reproduces the verified prefix
                        # sums. Codebook[0] enters via the ones tile.
                        acc = ppool.tile([P, PACKB], mybir.dt.float32,
                                         tag=f"ps{half}")
                        mm = [(dt_, None) for dt_ in diag_tiles[0]]
                        for k in range(1, 16):
                            if not diag_tiles[k]:
                                continue
                            m = wpool.tile([P, PACKB], mybir.dt.bfloat16,
                                           tag=f"mb{half}")
                            nc.vector.tensor_scalar(
                                out=m[:], in0=src, scalar1=float(16.0 * k),
                                scalar2=None, op0=OP.is_ge,
                            )
                            mm.extend((dt_, m) for dt_ in diag_tiles[k])
                        for j, (dt_, m) in enumerate(mm):
                            rhs = ones_bf[:] if m is None else m[:]
                            nc.tensor.matmul(out=acc[:], lhsT=dt_[:], rhs=rhs,
                                             start=(j == 0),
                                             stop=(j == len(mm) - 1))
                    else:
                        # fp32 mask*d terms on the DVE, accumulated by Pool
                        # tensor_tensor adds (the only tensor op in Pool ucode)
                        ms = []
                        for k in range(1, 16):
                            m = wpool.tile([P, PACKB], mybir.dt.float32,
                                           tag=f"m{half}")
                            nc.vector.tensor_scalar(
                                out=m[:], in0=src,
                                scalar1=float(16.0 * k), scalar2=float(d_k[k - 1]),
                                op0=OP.is_ge, op1=OP.mult,
                            )
                            ms.append(m)
                        acc = apool.tile([P, PACKB], mybir.dt.float32,
                                         tag=f"acc{half}")
                        nc.gpsimd.tensor_tensor(
                            out=acc[:], in0=ms[0][:], in1=c0_tile[:], op=OP.add
                        )
                        for m in ms[1:]:
                            nc.gpsimd.tensor_tensor(
                                out=acc[:], in0=acc[:], in1=m[:], op=OP.add
                            )
                    # interleaved write with the exact fp32 absmax scale
                    nc.scalar.activation(
                        out=out_t[:, half:D:2], in_=acc[:],
                        func=AF.Copy, bias=0.0, scale=a_t[:, 0:1],
                    )
                nc.sync.dma_start(out=out_d[i * P : (i + 1) * P, :], in_=out_t[:])

    if split_waits:
        # needed for walrus codegen; CoreSim's race detector rejects the
        # synthetic NoOps, so simulator-based tests build with False
        _split_wait_heavy(nc)
    return nc


_CACHE: dict = {}


def _get_nc(codebook: np.ndarray):
    key = codebook.astype(np.float32).tobytes()
    if key not in _CACHE:
        _CACHE[key] = build_kernel(codebook)
    return _CACHE[key]


def kernel(x, packed, absmax, codebook) -> np.ndarray:
    x = np.asarray(x)
    packed = np.asarray(packed, dtype=np.uint8)
    absmax = np.ascontiguousarray(absmax, dtype=np.float32)
    codebook = np.asarray(codebook, dtype=np.float32)
    assert x.shape == (B, S) and packed.shape == (V, PACKB) and absmax.shape == (V,)

    # table layout prep: append each row's absmax so one gather fetches both
    tbl = np.empty((V, ROWB), dtype=np.uint8)
    tbl[:, :PACKB] = packed
    tbl[:, PACKB:] = absmax.view(np.uint8).reshape(V, 4)

    idx = np.ascontiguousarray(x.astype(np.int32))  # [8, 4096] -> one row per core

    nc = _get_nc(codebook)
    in_maps = [{"idx": idx[c], "tbl": tbl} for c in range(N_CORES)]
    res = run_bass_kernel_spmd(nc, in_maps, core_ids=list(range(N_CORES)))
    out = np.stack([res.results[c]["out"] for c in range(N_CORES)], axis=0)
    return out.astype(np.float32, copy=False)


# revision 31
# speedup vs baseline: 74.7871x; 1.0017x over previous
"""NF4 (bitsandbytes-style) 4-bit quantized embedding lookup on 8 TRN2 NeuronCores.

Reference semantics (per token t with id x_t):
    row   = packed[x_t]                      # [512] uint8, two nf4 codes per byte
    hi    = row >> 4 ; lo = row & 0xF        # nibbles, even/odd output positions
    out_t = codebook[interleave(hi, lo)] * absmax[x_t]   # [1024] float32

Sharding: data-parallel over the batch dim (8 batch rows == 8 cores, 4096
tokens each). The 25 MB table is replicated per core; token rows are fetched
with an indirect (gather) DMA.

Decode strategy (exact fp32): the sorted codebook is evaluated as a staircase
    c[q] = c0 + sum_{k=1..15} d_k * (q >= k)
with increments d_k chosen so every fp32 prefix sum lands exactly on c[k]
(verified at build time). The hi-nibble staircase runs directly on raw bytes
(q_hi >= k  <=>  byte >= 16k); the lo staircase runs on (byte & 15) << 4 so
both share thresholds 16k. Each per-term mask is one DVE
tensor_scalar op; term accumulation chains are load-balanced across the
vector engine (fused mask*d+acc), GPSIMD (tensor_tensor adds), and the
tensor engine (PSUM chains of diag(piece) @ mask matmuls, with increments
pre-split into bf16-exact pieces so every product and add is exact) —
all bit-exact.
The per-row absmax scale (gathered inline as 4 extra bytes per table row)
is applied by the scalar engine while writing the interleaved halves of the
output tile.
"""

import numpy as np

try:
    import concourse.bass as bass
except ImportError:  # pragma: no cover - path fallback for bare containers
    import sys

    sys.path.insert(0, "/opt/trn_rl_repo")
    import concourse.bass as bass

import concourse.tile as tile
from concourse import mybir
from concourse.bass import IndirectOffsetOnAxis
from concourse.bass_utils import run_bass_kernel_spmd

V, D = 50257, 1024
B, S = 8, 4096
PACKB = D // 2          # packed bytes per row
ROWB = PACKB + 4        # + absmax fp32 appended per row
P = 128                 # SBUF partitions (tokens per tile)
N_TOK = S               # tokens per core
NT = N_TOK // P         # tiles per core
N_CORES = 8

AF = mybir.ActivationFunctionType
OP = mybir.AluOpType


def _staircase_increments(codebook: np.ndarray) -> np.ndarray:
    """d_k such that the fp32 running sum c0 + d1 + ... + d_k == codebook[k]
    exactly, for every prefix (nudged by ULPs if plain differences round)."""
    c = codebook.astype(np.float32)
    assert c.shape == (16,)
    assert np.all(np.diff(c.astype(np.float64)) > 0), "codebook must be sorted"
    ds = []
    run = c[0]
    for k in range(1, 16):
        d = np.float32(c[k] - run)
        if np.float32(run + d) != c[k]:
            for _ in range(8):
                d = np.nextafter(d, np.float32(c[k] - run), dtype=np.float32)
                if np.float32(run + d) == c[k]:
                    break
        assert np.float32(run + d) == c[k], f"cannot hit codebook[{k}] exactly"
        ds.append(float(d))
        run = np.float32(run + d)
    return np.array(ds, dtype=np.float32)


def _staircase_bf16_pieces(codebook: np.ndarray) -> list[list[float]]:
    """Split each staircase increment into <=4 bf16-exact pieces such that the
    fp32 running sum (starting at 0, pieces added in order) lands exactly on
    codebook[k] after k's pieces. bf16 x bf16 products are exact in fp32, so
    a PE matmul chain of diag(piece) @ mask accumulates these bit-exactly."""
    import ml_dtypes

    def bf16(x):
        return np.float32(ml_dtypes.bfloat16(np.float32(x)))

    c = codebook.astype(np.float32)
    out: list[list[float]] = []
    S = np.float32(0.0)
    for k in range(16):
        pieces: list[np.float32] = []
        for _ in range(4):
            if S == c[k]:
                break
            p = bf16(np.float32(c[k] - S))
            if p == 0.0:
                break
            pieces.append(p)
            S = np.float32(S + p)
        if S != c[k]:
            for _ in range(8):
                last = pieces[-1]
                Sbase = np.float32(S - last)
                b = ml_dtypes.bfloat16(last)
                bn = np.nextafter(b, ml_dtypes.bfloat16(np.float32(c[k]) - Sbase))
                pieces[-1] = np.float32(bn)
                S = np.float32(Sbase + pieces[-1])
                if S == c[k]:
                    break
                p = bf16(np.float32(c[k] - S))
                if p != 0.0:
                    pieces.append(p)
                    S = np.float32(S + p)
                if S == c[k]:
                    break
            assert S == c[k], f"bf16 piece split failed at codebook[{k}]"
        out.append([float(p) for p in pieces])
    # verify every prefix
    S = np.float32(0.0)
    for k in range(16):
        for p in out[k]:
            S = np.float32(S + p)
        assert S == c[k]
    return out


_MAX_WAITS = 1  # walrus setupSyncWait rejects instructions with too many waits


def _split_wait_heavy(nc, maxw: int = _MAX_WAITS):
    """Walrus caps the number of semaphore waits a single instruction may
    carry; Tile's kernel-tail drain can exceed it (one wait per DMA sem lane
    still unobserved by the sync engine). Splitting excess waits onto
    preceding same-engine NoOps is semantically identical — a sequencer
    executes its instructions in order, so the waits still all happen
    before the original instruction issues."""
    n = 0
    for fn in nc.m.functions:
        for bb in fn.blocks:
            il = bb.instructions
            if not any(
                i.sync_info is not None and len(i.sync_info.on_wait) > maxw
                for i in il
            ):
                continue
            out = []
            for ins in il:
                si = ins.sync_info
                if si is not None and len(si.on_wait) > maxw:
                    waits = list(si.on_wait)
                    while len(waits) > maxw:
                        chunk, waits = waits[:maxw], waits[maxw:]
                        n += 1
                        out.append(
                            mybir.InstNoOp(
                                name=f"WSPLIT-{n}",
                                engine=ins.engine,
                                bass_nofuse=True,
                                sync_info=mybir.SyncInfo(
                                    on_wait=chunk, on_update=[]
                                ),
                            )
                        )
                    ins.sync_info = mybir.SyncInfo(
                        on_wait=waits, on_update=list(si.on_update)
                    )
                out.append(ins)
            bb.instructions = out


def build_kernel(codebook: np.ndarray, n_tok: int = N_TOK, vocab: int = V,
                 tune: dict | None = None, split_waits: bool = True):
    """Trace the per-core Bass program (SPMD: same program, per-core inputs)."""
    t = {"g_bufs": 10, "m_bufs": 16, "acc_bufs": 4, "ps_bufs": 3, "out_bufs": 4,
         "cost_dve": 11000.0, "cost_pe": 8000.0, "cost_pool": 17000.0}
    if tune:
        t.update(tune)
    d_k = _staircase_increments(codebook)
    pieces = _staircase_bf16_pieces(codebook)
    c0 = float(np.float32(codebook[0]))
    nt = n_tok // P

    nc = bass.Bass()
    idx_d = nc.declare_dram_parameter("idx", [n_tok], mybir.dt.int32, isOutput=False)
    tbl_d = nc.declare_dram_parameter("tbl", [vocab, ROWB], mybir.dt.uint8, isOutput=False)
    out_d = nc.declare_dram_parameter("out", [n_tok, D], mybir.dt.float32, isOutput=True)

    from concourse.masks import make_identity

    # Greedy per-chain engine balancer over cost-model ns. A chain placed on
    # engine E costs E its accumulate work AND costs the DVE the 15 mask
    # terms (bf16 4x masks for DVE chains, fp32 2x mask*d for PE/Pool).
    CH_COST = {"dve": t["cost_dve"], "pe": t["cost_pe"], "pool": t["cost_pool"]}
    MASK_COST = {"dve": 2950.0, "pe": 2950.0, "pool": 4950.0}
    load = {"dve": 0.0, "pe": 0.0, "pool": 35000.0}  # pool pre-load: descgen

    def pick_chain_engine():
        best, best_peak = None, None
        for e in CH_COST:
            trial = dict(load)
            trial[e] += CH_COST[e]
            trial["dve"] += MASK_COST[e]
            peak = max(trial.values())
            if best_peak is None or peak < best_peak:
                best, best_peak = e, peak
        load[best] += CH_COST[best]
        load["dve"] += MASK_COST[best]
        return best

    with tile.TileContext(nc) as tc:
        with (
            tc.tile_pool(name="const", bufs=1) as const_pool,
            tc.tile_pool(name="gather", bufs=t["g_bufs"]) as gpool,
            tc.tile_pool(name="work", bufs=t["m_bufs"]) as wpool,
            tc.tile_pool(name="acc", bufs=t["acc_bufs"]) as apool,
            tc.tile_pool(name="psum", bufs=t["ps_bufs"], space="PSUM") as ppool,
            tc.tile_pool(name="mwide", bufs=t.get("mw_bufs", 16)) as mpool2,
            tc.tile_pool(name="outp", bufs=t["out_bufs"]) as opool,
        ):
            # all token ids, one small DMA: SBUF [P, nt], column i = tile i
            idx_sb = const_pool.tile([P, nt], mybir.dt.int32)
            nc.sync.dma_start(
                out=idx_sb[:], in_=idx_d[:].rearrange("(n p) -> p n", p=P)
            )
            # staircase base: acc starts at codebook[0]
            c0_tile = const_pool.tile([P, PACKB], mybir.dt.float32)
            nc.vector.memset(c0_tile[:], c0)
            ident_bf = const_pool.tile([P, P], mybir.dt.bfloat16)
            make_identity(nc, ident_bf[:])
            ones_bf = const_pool.tile([P, PACKB], mybir.dt.bfloat16)
            nc.vector.memset(ones_bf[:], 1.0)
            # diag(piece) lhsT tiles, built by the (otherwise idle) scalar
            # engine: Copy with an fp32-imm scale of a bf16-exact value
            # round-trips exactly through the bf16 output
            diag_tiles = []
            for k in range(16):
                row = []
                for p in pieces[k]:
                    dt_ = const_pool.tile([P, P], mybir.dt.bfloat16,
                                          tag=f"diag{len(diag_tiles)}_{len(row)}")
                    nc.scalar.activation(out=dt_[:], in_=ident_bf[:],
                                         func=AF.Copy, bias=0.0, scale=float(p))
                    row.append(dt_)
                diag_tiles.append(row)

            for i in range(nt):
                # gather into a [P, 1024] tile; after the absmax tail is
                # copied out, the lo-nibble extract overwrites bytes 512..1023
                # so both halves' mask source is one dense [P, 1024] block
                g = gpool.tile([P, 2 * PACKB], mybir.dt.uint8, tag="g")
                nc.gpsimd.indirect_dma_start(
                    out=g[:, 0:ROWB],
                    out_offset=None,
                    in_=tbl_d[:, :],
                    in_offset=IndirectOffsetOnAxis(ap=idx_sb[:, i : i + 1], axis=0),
                )
                by = g[:, 0:PACKB]                       # raw bytes: hi staircase
                lo = g[:, PACKB : 2 * PACKB]             # (byte & 15) << 4
                # absmax must leave before the extract overwrites the tail
                a_t = wpool.tile([P, 1], mybir.dt.float32, tag="a")
                nc.any.tensor_copy(
                    out=a_t[:], in_=g[:, PACKB:ROWB].bitcast(mybir.dt.float32)
                )
                nc.any.tensor_scalar(
                    out=lo, in0=by, scalar1=15, scalar2=4,
                    op0=OP.bitwise_and, op1=OP.logical_shift_left,
                )
                gsrc = g[:, 0 : 2 * PACKB]

                out_t = opool.tile([P, D], mybir.dt.float32, tag="out")
                engs = (pick_chain_engine(), pick_chain_engine())
                merged = all(e in ("dve", "pe") for e in engs)
                if merged:
                    # one bf16 is_ge op per threshold covers both halves;
                    # each chain consumes its contiguous 512-col slice
                    accs, prevs, pe_j = {}, {}, {}
                    n_pe_ops = {h: 0 for h in (0, 1)}
                    total_pieces = sum(len(pc) for pc in pieces)
                    for h in (0, 1):
                        if engs[h] == "pe":
                            acc = ppool.tile([P, PACKB], mybir.dt.float32,
                                             tag=f"ps{h}")
                            for j, dt_ in enumerate(diag_tiles[0]):
                                nc.tensor.matmul(out=acc[:], lhsT=dt_[:],
                                                 rhs=ones_bf[:],
                                                 start=(j == 0), stop=False)
                            pe_j[h] = len(diag_tiles[0])
                        else:
                            acc = apool.tile([P, PACKB], mybir.dt.float32,
                                             tag=f"acc{h}")
                            prevs[h] = c0_tile
                        accs[h] = acc
                    for k in range(1, 16):
                        if not diag_tiles[k]:
                            continue
                        m2 = mpool2.tile([P, 2 * PACKB], mybir.dt.bfloat16,
                                         tag="mb2")
                        nc.vector.tensor_scalar(
                            out=m2[:], in0=gsrc, scalar1=float(16.0 * k),
                            scalar2=None, op0=OP.is_ge,
                        )
                        for h in (0, 1):
                            sl = m2[:, h * PACKB : (h + 1) * PACKB]
                            if engs[h] == "pe":
                                for dt_ in diag_tiles[k]:
                                    pe_j[h] += 1
                                    nc.tensor.matmul(
                                        out=accs[h][:], lhsT=dt_[:], rhs=sl,
                                        start=False,
                                        stop=(pe_j[h] == total_pieces),
                                    )
                            else:
                                nc.vector.scalar_tensor_tensor(
                                    out=accs[h][:], in0=sl,
                                    scalar=float(d_k[k - 1]),
                                    in1=prevs[h][:], op0=OP.mult, op1=OP.add,
                                )
                                prevs[h] = accs[h]
                    for h in (0, 1):
                        nc.scalar.activation(
                            out=out_t[:, h:D:2], in_=accs[h][:],
                            func=AF.Copy, bias=0.0, scale=a_t[:, 0:1],
                        )
                    nc.sync.dma_start(out=out_d[i * P : (i + 1) * P, :],
                                      in_=out_t[:])
                    continue

                for half, src_ap in ((0, by), (1, lo)):
                    eng = engs[half]
                    if eng == "dve":
                        acc = apool.tile([P, PACKB], mybir.dt.float32,
                                         tag=f"acc{half}")
                        prev = c0_tile
                        for k in range(1, 16):
                            m = wpool.tile([P, PACKB], mybir.dt.bfloat16,
                                           tag=f"mb{half}")
                            nc.vector.tensor_scalar(
                                out=m[:], in0=src_ap, scalar1=float(16.0 * k),
                                scalar2=None, op0=OP.is_ge,
                            )
                            nc.vector.scalar_tensor_tensor(
                                out=acc[:], in0=m[:], scalar=float(d_k[k - 1]),
                                in1=prev[:], op0=OP.mult, op1=OP.add,
                            )
                            prev = acc
                    elif eng == "pe":
                        acc = ppool.tile([P, PACKB], mybir.dt.float32,
                                         tag=f"ps{half}")
                        mm = [(dt_, None) for dt_ in diag_tiles[0]]
                        for k in range(1, 16):
                            if not diag_tiles[k]:
                                continue
                            m = wpool.tile([P, PACKB], mybir.dt.bfloat16,
                                           tag=f"mb{half}")
                            nc.vector.tensor_scalar(
                                out=m[:], in0=src_ap, scalar1=float(16.0 * k),
                                scalar2=None, op0=OP.is_ge,
                            )
                            mm.extend((dt_, m) for dt_ in diag_tiles[k])
                        for j, (dt_, m) in enumerate(mm):
                            rhs = ones_bf[:] if m is None else m[:]
                            nc.tensor.matmul(out=acc[:], lhsT=dt_[:], rhs=rhs,
                                             start=(j == 0),
                                             stop=(j == len(mm) - 1))
                    else:
                        ms = []
                        for k in range(1, 16):
                            m = wpool.tile([P, PACKB], mybir.dt.float32,
                                           tag=f"m{half}")
                            nc.vector.tensor_scalar(
                                out=m[:], in0=src_ap,
                                scalar1=float(16.0 * k), scalar2=float(d_k[k - 1]),
                                op0=OP.is_ge, op1=OP.mult,
                            )
                            ms.append(m)
                        acc = apool.tile([P, PACKB], mybir.dt.float32,
                                         tag=f"acc{half}")
                        nc.gpsimd.tensor_tensor(
                            out=acc[:], in0=ms[0][:], in1=c0_tile[:], op=OP.add
                        )
                        for m in ms[1:]:
                            nc.gpsimd.tensor_tensor(
                                out=acc[:], in0=acc[:], in1=m[:], op=OP.add
                            )
                    nc.scalar.activation(
                        out=out_t[:, half:D:2], in_=acc[:],
                        func=AF.Copy, bias=0.0, scale=a_t[:, 0:1],
                    )
                nc.sync.dma_start(out=out_d[i * P : (i + 1) * P, :], in_=out_t[:])

    if split_waits:
        # needed for walrus codegen; CoreSim's race detector rejects the
        # synthetic NoOps, so simulator-based tests build with False
        _split_wait_heavy(nc)
    return nc


_CACHE: dict = {}


def _get_nc(codebook: np.ndarray):
    key = codebook.astype(np.float32).tobytes()
    if key not in _CACHE:
        _CACHE[key] = build_kernel(codebook)
    return _CACHE[key]


def kernel(x, packed, absmax, codebook) -> np.ndarray:
    x = np.asarray(x)
    packed = np.asarray(packed, dtype=np.uint8)
    absmax = np.ascontiguousarray(absmax, dtype=np.float32)
    codebook = np.asarray(codebook, dtype=np.float32)
    assert x.shape == (B, S) and packed.shape == (V, PACKB) and absmax.shape == (V,)

    # table layout prep: append each row's absmax so one gather fetches both
    tbl = np.empty((V, ROWB), dtype=np.uint8)
    tbl[:, :PACKB] = packed
    tbl[:, PACKB:] = absmax.view(np.uint8).reshape(V, 4)

    idx = np.ascontiguousarray(x.astype(np.int32))  # [8, 4096] -> one row per core

    nc = _get_nc(codebook)
    in_maps = [{"idx": idx[c], "tbl": tbl} for c in range(N_CORES)]
    res = run_bass_kernel_spmd(nc, in_maps, core_ids=list(range(N_CORES)))
    out = np.stack([res.results[c]["out"] for c in range(N_CORES)], axis=0)
    return out.astype(np.float32, copy=False)
